# revision 69
# baseline (speedup 1.0000x reference)
"""Trainium2 Bass kernel for capsule attention-routing.

Reference computation (per pixel; 4096 independent problems of shape
[I=32 in-caps, N=32 out-caps, J=16 caps-dim]):
    v[n,j]   = sum_i u[i,n,j]
    cp[i,n]  = sum_j u[i,n,j] * v[n,j] / 4
    c[i,n]   = softmax_n(cp)[i,n] + b[i,n]
    s[n,j]   = sum_i u[i,n,j] * c[i,n]
    out[n,j] = (1 - 1/(exp(|s|_j)+eps)) * s[n,j] / (|s|_j + eps)

Sharding: data-parallel over (batch, h-half): 8 cores x 512 pixels.
Per-core: 8 blocks of 64 pixels, SBUF partitions = (j*8+il), il=i%8.

All tensors stream in fp16. Versus the pure-PE baseline (4 full u-passes on
the TensorEngine, 138us), this version spreads the broadcast/reduction work
across PE, DMA, DVE, Act and Pool so no engine carries more than ~11us/block:
  - v-pass: DVE presums u over one ib-pair (u0+u1), so the PE v-pass streams
    3 groups instead of 4 (2560ns vs 3413ns per block).
  - c-broadcast: only units ib0 (4 of 16) broadcast on the PE; the other 12
    units go via DMA: c_sb is bounced to DRAM (its rows are laid out
    (ib,il,q) so each (ib,il) is one contiguous 2048-elem DRAM row), then a
    single dma_gather with a constant int16 index table broadcasts rows to
    all 128 partitions (384 descriptors x 4KB, fits the 1024-desc SWDGE
    ring). This removes 2/3 of the old cbc PE pass AND the corresponding
    PSUM->SBUF Act copies. NOTE: the HW gather reads the idx table from a
    different partition window than bass_interp - the table must be the
    16-row wrapped pattern REPLICATED to all 128 partitions.
  - s-reduce is split into two PSUM accumulators: PE-half units (sp1) close
    one iteration after cp, DMA-half units (sp2) a further iteration later,
    giving the bounce+gather chain (~9us) a full extra iteration of slack;
    s = park1(sp1,f16) + sp2 is combined by one DVE add at drain time.
  - The m=u*c multiply: DMA-half units run as one wide op per ib group
    (ib1,ib2 on DVE 2x, ib3 on Pool); PE-half units copy cb via Act then
    multiply on DVE/Pool per _POOL_M_UNITS.
  - softmax z/rz/cmult all stay on DVE: the c_sb->bounce->gather chain is
    latency-critical and any hop through the (bursty) Pool queue adds ~4us
    per block (Pool runs 4us-wide m-group ops that block small ops).
  - squash uses ln/exp only (norm = exp(0.5 ln n2), rn = exp(-0.5 ln n2),
    en = exp(-norm)) so a single Act LUT set is loaded once - no per-pair
    sqrt<->exp table flips (saves 12 x 1283ns of Act serial time).
  - u loads issue in-loop (u(i+3) after bounce(i)) so their tile-pool-slot
    waits never block the bounce/gather DMAs queued behind them on SP.
Softmax runs without max-subtraction (|cp| <~ 45 safe in fp32 exp; e kept
f32 in SBUF since exp overflows fp16).
Engine busy per 128us run: PE 83us, DVE 87us, DMA 89us (at its volume
floor: 16KB u + 12KB gather + 1.5KB bounce/out per partition per block),
Act 62us, Pool 62us.
"""

import math
import numpy as np
from contextlib import ExitStack

import concourse.bass as bass
import concourse.bacc as bacc
import concourse.tile as tile
import concourse.mybir as mybir
from concourse.bass_utils import run_bass_kernel_spmd

dt = mybir.dt
AF = mybir.ActivationFunctionType
OP = mybir.AluOpType

B, I, N, J, H, W = 4, 32, 32, 16, 32, 32
HW = H * W
NCORES = 8
PIX = B * HW // NCORES      # 512 pixels per core
BLK = 64                    # pixels per block
P16, P8 = 16, 8
SCALE = 1.0 / math.sqrt(16.0)     # 0.25

f32, bf16, f32r = dt.float32, dt.bfloat16, dt.float32r
f16 = dt.float16

# m-stage split per block: units 0..7 get their c-broadcast from the PE
# (PSUM -> Act copy -> SBUF f16), units 8..15 from a DMA doubling-broadcast
# (straight to SBUF f16). The elementwise m-mult splits DVE/Pool per unit.
_POOL_M_UNITS = {1, 3}
N_PE_CBC = 4  # units 0..N_PE_CBC-1 broadcast on PE, rest via DMA


def _build_weight_arrays():
    il_of = np.arange(128) % 8          # partition -> il
    j_of = np.arange(128) // 8          # partition -> j

    # v-pass: out[(j2,il2)] = sum_il u[(j,il)] for j==j2 (broadcast over il2)
    wv = np.zeros((128, 128), np.float32)
    for p_in in range(128):
        for p_out in range(128):
            if j_of[p_in] == j_of[p_out]:
                wv[p_in, p_out] = 1.0

    # c-reduce: 16 blocks k=q*4+ib: out row (ib,il,q) = ib*32+il*4+q
    # (row order makes each (ib,il) a contiguous 2048-elem row in the DRAM
    # bounce, so the cbc gather uses 384 4KB descriptors)
    wc = np.zeros((128, 16 * 128), np.float32)
    for q in range(4):
        for ib in range(4):
            k = q * 4 + ib
            for p_in in range(128):
                wc[p_in, k * 128 + ib * 32 + il_of[p_in] * 4 + q] = SCALE

    # c-bcast (PE-half units = ib 0): moving rows 0..32 = (il,q'), strip for
    # unit q: col (j*8+il): delta(q'==q), any j
    wcb = np.zeros((128, 4 * 128), np.float32)
    for q in range(4):
        for il in range(8):
            for j in range(16):
                wcb[il * 4 + q, q * 128 + j * 8 + il] = 1.0

    # s-reduce: 8 blocks q8: out[q8*16+j2] = sum_il m[(j,il)] with j==j2
    ws = np.zeros((128, 8 * 128), np.float32)
    for q8 in range(8):
        for p_in in range(128):
            ws[p_in, q8 * 128 + q8 * 16 + j_of[p_in]] = 1.0

    # norm2: out[(q8b*16+r)] = sum_j ssq[(q8*16+j)] for q8==q8b
    wn = np.zeros((128, 128), np.float32)
    for p_in in range(128):
        for p_out in range(128):
            if p_in // 16 == p_out // 16:
                wn[p_in, p_out] = 1.0

    return {"wv": wv, "wc": wc, "wcb": wcb, "ws": ws, "wn": wn}


def _gidx_array():
    """Replicated-wrap int16 index table for the cbc gather: idx k lives at
    [k % 16, k // 16], replicated across all 128 partitions (HW reads the
    table from a different partition window than the interp)."""
    ng = (16 - N_PE_CBC) // 4
    idxs = np.zeros(ng * 128, np.int16)
    for g in range(ng):
        ib = g + N_PE_CBC // 4
        for r in range(128):
            idxs[g * 128 + r] = ib * 8 + (r % 8)
    a = np.zeros((16, ng * 128 // 16), np.int16)
    for k in range(ng * 128):
        a[k % 16, k // 16] = idxs[k]
    return np.tile(a, (8, 1))


def _b_tile_array(b_np):
    # b_t[q*32+ib*8+il, n*16+p] = b[0, ib*8+il, n, 0,0,0]
    bt = np.zeros((128, N * P16), np.float32)
    bsl = b_np.reshape(I, N)
    for q in range(4):
        for ib in range(4):
            for il in range(8):
                row = ib * 32 + il * 4 + q
                bt[row, :] = np.repeat(bsl[ib * 8 + il, :], P16)
    return bt


class _Block:
    """Per-block live tiles (filled in as stages emit)."""

    def __init__(self):
        self.u = None        # [128, (ib,n,p64)] f16 SBUF
        self.v_sb = None     # [128, (n,p64)] f16 SBUF
        self.w = None        # [128, (ib,n,p64)] f16 SBUF
        self.c_ps = None     # [128, (n,p16)] f32 PSUM
        self.e_sb = None     # [128, (n,p16)] f32 SBUF
        self.c_sb = None     # [128, (n,p16)] f16 SBUF
        self.m = [None] * 16  # per (ib,q) unit: [128, (n,p16)] f16 SBUF
        self.sp1 = None      # [128, (n,p8)] f32 PSUM (PE-half s partial)
        self.sp2 = None      # [128, (n,p8)] f32 PSUM (DMA-half s partial)
        self.park1 = None    # [128, (n,p8)] f16 SBUF
        self.ssq = None
        self.n2 = None
        self.norm = None
        self.en = None
        self.rn = None


def _emit(ctx: ExitStack, tc: tile.TileContext, aps: dict, pix: int, with_b: bool):
    nc = tc.nc
    nblk = pix // BLK
    u_d, o_d = aps["u"], aps["out"]

    # constants
    pconst = ctx.enter_context(tc.tile_pool(name="const", bufs=1))
    wv_t = pconst.tile([128, 128], f16, tag="wv")
    wc_t = pconst.tile([128, 16 * 128], f16, tag="wc")
    wcb_t = pconst.tile([128, 4 * 128], f16, tag="wcb")
    ws_t = pconst.tile([128, 8 * 128], f16, tag="ws")
    wn_t = pconst.tile([128, 128], f32r, tag="wn")
    gidx_t = pconst.tile([128, (16 - N_PE_CBC) // 4 * 128 // 16], dt.int16, tag="gidx")
    if with_b:
        bt_t = pconst.tile([128, N * P16], f32, tag="bt")

    # pools
    pu = ctx.enter_context(tc.tile_pool(name="u", bufs=4))
    pw = ctx.enter_context(tc.tile_pool(name="w", bufs=2))
    pvp = ctx.enter_context(tc.tile_pool(name="vp", bufs=1))
    pvsb = ctx.enter_context(tc.tile_pool(name="vsb", bufs=2))
    pesb = ctx.enter_context(tc.tile_pool(name="esb", bufs=2))
    psmall = ctx.enter_context(tc.tile_pool(name="small", bufs=2))
    pcbsb = ctx.enter_context(tc.tile_pool(name="cbsb", bufs=N_PE_CBC + 2))
    pcbd = ctx.enter_context(tc.tile_pool(name="cbd", bufs=2))
    pdram = ctx.enter_context(tc.tile_pool(name="cdram", bufs=2, space="DRAM"))
    pm = ctx.enter_context(tc.tile_pool(name="m", bufs=10))
    pmg = ctx.enter_context(tc.tile_pool(name="mg", bufs=3))
    psq = ctx.enter_context(tc.tile_pool(name="sq", bufs=2))
    psqp = ctx.enter_context(tc.tile_pool(name="sqp", bufs=2))

    pvps = ctx.enter_context(tc.tile_pool(name="vps", bufs=2, space="PSUM"))
    pcps = ctx.enter_context(tc.tile_pool(name="cps", bufs=1, space="PSUM"))
    pcb = ctx.enter_context(tc.tile_pool(name="cb", bufs=2, space="PSUM"))
    psp1 = ctx.enter_context(tc.tile_pool(name="sp1", bufs=1, space="PSUM"))
    psp2 = ctx.enter_context(tc.tile_pool(name="sp2", bufs=1, space="PSUM"))
    pn2 = ctx.enter_context(tc.tile_pool(name="n2", bufs=1, space="PSUM"))
    ppark = ctx.enter_context(tc.tile_pool(name="park", bufs=2))

    blocks = [_Block() for _ in range(nblk)]

    # ---- stage emitters ----

    def dma_u(i, chunked=False, first_n=None, rest=False):
        if rest:
            t = blocks[i].u
        else:
            t = pu.tile([128, 4 * N * BLK], f16, tag="T", name="T")
            blocks[i].u = t
        if chunked:
            # v-stage-granular chunks so early blocks' v-pass starts sooner
            u4 = u_d[i].rearrange("P (ib n p) -> P ib n p", ib=4, p=BLK)
            t4 = t[:].rearrange("P (ib n p) -> P ib n p", ib=4, p=BLK)
            width = 8 if chunked is True else 16
            nch = 32 // width
            sts = range(nch) if not rest else range(1, nch)
            for st in sts:
                nc.sync.dma_start(
                    t4[:, :, st * width : (st + 1) * width, :],
                    u4[:, :, st * width : (st + 1) * width, :],
                )
        else:
            nc.sync.dma_start(t[:], u_d[i])

    def dve_vp(i, nchunks=1):
        """single-pair ib-presum of u on DVE: vp = u0+u1 (u2,u3 fed to the
        v-pass unsummed). Chunked along n so early chunks unblock pe_v."""
        bl = blocks[i]
        vp = pvp.tile([128, N * BLK], f16, tag="vp", name="vp_t")
        u3 = bl.u[:].rearrange("P (ib c) -> P ib c", ib=4)
        w = (N * BLK) // nchunks
        for ck in range(nchunks):
            sl = slice(ck * w, (ck + 1) * w)
            nc.vector.tensor_tensor(vp[:, sl], u3[:, 0, sl], u3[:, 1, sl], op=OP.add)
        bl.vp = vp

    def pe_v(i):
        """v-pass on [vp01, u2, u3] + Act copies to v_sb (f16)."""
        bl = blocks[i]
        u3 = bl.u[:].rearrange("P (ib c) -> P ib c", ib=4)
        v_sb = pvsb.tile([128, N * BLK], f16, tag="vsb")
        for st in range(4):
            sl = slice(st * 512, (st + 1) * 512)
            v_ps = pvps.tile([128, 512], f32, tag="vps")
            nc.tensor.matmul(v_ps[:], wv_t[:], bl.vp[:, sl], start=True, stop=False)
            nc.tensor.matmul(v_ps[:], wv_t[:], u3[:, 2, sl], start=False, stop=False)
            nc.tensor.matmul(v_ps[:], wv_t[:], u3[:, 3, sl], start=False, stop=True)
            nc.scalar.copy(v_sb[:, sl], v_ps[:])
        bl.v_sb = v_sb

    def pe_cp(i):
        """cp-reduce: 16 matmuls (ib-outer for early-w consumption)."""
        bl = blocks[i]
        c_ps = pcps.tile([128, N * P16], f32, tag="cps")
        c_ps_v = c_ps[:].rearrange("P (n p) -> P n p", p=P16)
        w4 = bl.w[:].rearrange("P (ib n p) -> P ib n p", ib=4, p=BLK)
        # split by n-half so each matmul only needs one w-chunk per ib
        for half in range(2):
            n_sl = slice(half * 16, (half + 1) * 16)
            for ib in range(4):
                for q in range(4):
                    nc.tensor.matmul(
                        c_ps_v[:, n_sl, :],
                        wc_t[:, (q * 4 + ib) * 128 : (q * 4 + ib + 1) * 128],
                        w4[:, ib, n_sl, q * P16 : (q + 1) * P16],
                        start=(ib == 0 and q == 0),
                        stop=(ib == 3 and q == 3),
                        skip_group_check=True,
                    )
        bl.c_ps = c_ps

    def act_exp(i):
        bl = blocks[i]
        e_sb = pesb.tile([128, N * P16], f32, tag="esb")
        nc.scalar.activation(e_sb[:], bl.c_ps[:], AF.Exp)
        bl.e_sb = e_sb

    def soft(i):
        """softmax normalize: z-sum + 1/z + e*rz all on DVE so the chain
        to the bounce stays on one engine (no cross-queue delays)."""
        bl = blocks[i]
        z = psmall.tile([128, P16], f32, tag="z")
        nc.vector.tensor_reduce(
            z[:],
            bl.e_sb[:].rearrange("P (n p) -> P p n", p=P16),
            axis=mybir.AxisListType.X,
            op=OP.add,
        )
        rz = psmall.tile([128, P16], f32, tag="rz")
        nc.vector.reciprocal(rz[:], z[:])
        c_sb = psmall.tile([128, N * P16], f16, tag="csb")
        rz_b = rz[:].rearrange("P (o p) -> P o p", o=1).broadcast_to([128, N, P16])
        if with_b:
            c_f = psmall.tile([128, N * P16], f32, tag="cf")
            nc.gpsimd.tensor_tensor(
                c_f[:].rearrange("P (n p) -> P n p", p=P16),
                bl.e_sb[:].rearrange("P (n p) -> P n p", p=P16),
                rz_b,
                op=OP.mult,
            )
            nc.gpsimd.tensor_tensor(c_sb[:], c_f[:], bt_t[:], op=OP.add)
        else:
            nc.vector.tensor_tensor(
                c_sb[:].rearrange("P (n p) -> P n p", p=P16),
                bl.e_sb[:].rearrange("P (n p) -> P n p", p=P16),
                rz_b,
                op=OP.mult,
            )
        bl.c_sb = c_sb

    def pe_cbc_unit(i, u_ix):
        """c-bcast matmul for unit (ib,q) -> cb PSUM tile."""
        bl = blocks[i]
        ib, q = divmod(u_ix, 4)
        cb = pcb.tile([128, N * P16], f32, tag="cb")
        nc.tensor.matmul(
            cb[:].rearrange("P (n p) -> P n p", p=P16),
            wcb_t[0:32, q * 128 : (q + 1) * 128],
            bl.c_sb[0:32, :].rearrange("P (n p) -> P n p", p=P16),
            start=True,
            stop=True,
            skip_group_check=True,
            tile_position=(0, 0),
        )
        return cb

    def m_unit(i, u_ix, cb):
        """m[(ib,q)] = u-slice * cb (PE-half): Act PSUM copy + DVE/Pool mult."""
        bl = blocks[i]
        ib, q = divmod(u_ix, 4)
        u_sl = (
            bl.u[:]
            .rearrange("P (ib n p) -> P ib n p", ib=4, p=BLK)[
                :, ib, :, q * P16 : (q + 1) * P16
            ]
        )
        m = pm.tile([128, N * P16], f16, tag="m", name="m_u")
        cb_sb = pcbsb.tile([128, N * P16], f16, tag="cbsb", name="cb_sb")
        nc.scalar.copy(cb_sb[:], cb[:])
        eng = nc.gpsimd if u_ix in _POOL_M_UNITS else nc.vector
        eng.tensor_tensor(
            m[:].rearrange("P (n p) -> P n p", p=P16),
            u_sl,
            cb_sb[:].rearrange("P (n p) -> P n p", p=P16),
            op=OP.mult,
        )
        bl.m[u_ix] = m[:]

    def dma_cbc_bounce(i):
        """Stage c_sb to DRAM (SP queue) right after softmax finishes."""
        bl = blocks[i]
        cst = pdram.tile([128, N * P16], f16, tag="cst", name="cst_t")
        nc.sync.dma_start(cst[:], bl.c_sb[:])
        bl.cst = cst

    def dma_cbc_gather(i):
        """One gather DMA broadcasts c to all 128 rows for units N_PE_CBC..15:
        cbd[(j,il), (u', n, p16)] = c[(ib,il), (n, p16-of-q)], u'=(ib,q)-8."""
        bl = blocks[i]
        ng = (16 - N_PE_CBC) // 4
        cbd = pcbd.tile([128, ng * 4 * N * P16], f16, tag="cbd", name="cbd_t")
        nc.gpsimd.dma_gather(
            cbd[:].rearrange("P (g c) -> P g c", c=4 * N * P16),
            bl.cst[:].rearrange("(a b) c -> a (b c)", b=4),
            gidx_t[:],
            num_idxs=ng * 128,
            num_idxs_reg=ng * 128,
            elem_size=4 * N * P16,
        )
        bl.cbd = cbd

    def m_unit_dma_group(i, ib, eng):
        """merged m-mult for the 4 DMA-broadcast units of one ib:
        one wide op over (q, n, p16)."""
        bl = blocks[i]
        up0 = (ib - N_PE_CBC // 4) * 4
        u_sl = (
            bl.u[:]
            .rearrange("P (ib n q p) -> P ib q n p", ib=4, q=4, p=P16)[:, ib]
        )
        m = pmg.tile([128, 4 * N * P16], f16, tag="mg", name="m_g")
        m_v = m[:].rearrange("P (q n p) -> P q n p", q=4, p=P16)
        eng.tensor_tensor(
            m_v,
            u_sl,
            bl.cbd[:, up0 * 512 : (up0 + 4) * 512].rearrange(
                "P (q n p) -> P q n p", q=4, p=P16
            ),
            op=OP.mult,
        )
        for q in range(4):
            bl.m[ib * 4 + q] = m[:, q * 512 : (q + 1) * 512]

    def pe_s_unit(i, u_ix):
        """two s-reduce matmuls consuming m[u_ix] of block i.
        PE-half units (u<8) accumulate in sp1 (closed in iter i+1);
        DMA-half units in sp2 (closed in iter i+2, giving the c-broadcast
        gather a full extra iteration of slack)."""
        bl = blocks[i]
        ib, q = divmod(u_ix, 4)
        if u_ix < N_PE_CBC:
            if bl.sp1 is None:
                bl.sp1 = psp1.tile([128, N * P8], f32, tag="sp1", name="sp1_t")
            tgt = bl.sp1
            first, last = 0, N_PE_CBC - 1
        else:
            if bl.sp2 is None:
                bl.sp2 = psp2.tile([128, N * P8], f32, tag="sp2", name="sp2_t")
            tgt = bl.sp2
            first, last = N_PE_CBC, 15
        spk_v = tgt[:].rearrange("P (n p) -> P n p", p=P8)
        m_v = bl.m[u_ix].rearrange("P (n p) -> P n p", p=P16)
        for k2 in range(2):
            q8 = 2 * q + k2
            nc.tensor.matmul(
                spk_v,
                ws_t[:, q8 * 128 : (q8 + 1) * 128],
                m_v[:, :, k2 * P8 : (k2 + 1) * P8],
                start=(u_ix == first and k2 == 0),
                stop=(u_ix == last and k2 == 1),
                skip_group_check=True,
            )

    def act_park1(i):
        """park the PE-half s partial (frees sp1's PSUM bank)."""
        bl = blocks[i]
        bl.park1 = ppark.tile([128, N * P8], f16, tag="pk1", name="pk1_t")
        nc.scalar.copy(bl.park1[:], bl.sp1[:])

    def dve_stot(i):
        """s = park1 + sp2 into the pair's SBUF tile (frees sp2)."""
        bl = blocks[i]
        ev = blocks[i - i % 2]
        if i % 2 == 0:
            ev.s_sbp = psqp.tile([128, 2 * N * P8], f16, tag="s_sbp", name="s_sbp")
        nc.vector.tensor_tensor(
            ev.s_sbp[:, (i % 2) * N * P8 : (i % 2 + 1) * N * P8],
            bl.park1[:],
            bl.sp2[:],
            op=OP.add,
        )

    def act_square(i):
        bl = blocks[i]
        ev = blocks[i - i % 2]
        ssq = psq.tile([128, N * P8], f32r, tag="ssq")
        nc.scalar.activation(
            ssq[:], ev.s_sbp[:, (i % 2) * N * P8 : (i % 2 + 1) * N * P8], AF.Square
        )
        bl.ssq = ssq

    def pe_n2(i):
        bl = blocks[i]
        ev = blocks[i - i % 2]
        if i % 2 == 0:
            ev.n2p = pn2.tile([128, 2 * N * P8], f32, tag="n2p", name="n2p")
        nc.tensor.matmul(
            ev.n2p[:, (i % 2) * N * P8 : (i % 2 + 1) * N * P8],
            wn_t[:],
            bl.ssq[:],
            start=True,
            stop=True,
            skip_group_check=True,
        )

    def act_sqrt_pair(p, half=None):
        """t = ln(n2 + tiny); lands at iteration end (dep: n2 matmuls).
        Using ln (same act table set as exp) avoids all table flips.
        half=0/1 processes one block's slice (used to pipeline the drain)."""
        ev = blocks[2 * p]
        if ev.norm is None:
            ev.norm = psqp.tile([128, 2 * N * P8], f32, tag="norm", name="normp")
        sl = slice(0, 2 * N * P8) if half is None else slice(half * N * P8, (half + 1) * N * P8)
        nc.scalar.activation(ev.norm[:, sl], ev.n2p[:, sl], AF.Ln, bias=1e-30)

    def act_en_pair(p, half=None):
        """norm = exp(0.5 t), rn = 1/norm = exp(-0.5 t), en = exp(-norm)."""
        ev = blocks[2 * p]
        if ev.en is None:
            ev.en = psqp.tile([128, 2 * N * P8], f32, tag="en", name="enp")
            # nrm borrows the g tile: g is only written after en consumed it
            ev.g = psqp.tile([128, 2 * N * P8], f32, tag="g", name="gp")
            ev.nrm = ev.g
        sl = slice(0, 2 * N * P8) if half is None else slice(half * N * P8, (half + 1) * N * P8)
        nc.scalar.activation(ev.nrm[:, sl], ev.norm[:, sl], AF.Exp, scale=0.5)
        nc.scalar.activation(ev.en[:, sl], ev.nrm[:, sl], AF.Exp, scale=-1.0)

    def dve_rn_pair(p, half=None):
        ev = blocks[2 * p]
        if ev.rn is None:
            ev.rn = psqp.tile([128, 2 * N * P8], f32, tag="rn", name="rnp")
        sl = slice(0, 2 * N * P8) if half is None else slice(half * N * P8, (half + 1) * N * P8)
        nc.scalar.activation(ev.rn[:, sl], ev.norm[:, sl], AF.Exp, scale=-0.5)

    def pool_g_pair(p, last=False, half=None):
        ev = blocks[2 * p]
        if getattr(ev, "g", None) is None:
            ev.g = psqp.tile([128, 2 * N * P8], f32, tag="g", name="gp")
        sl = slice(0, 2 * N * P8) if half is None else slice(half * N * P8, (half + 1) * N * P8)
        # g = (en - 1) * rn, single DVE stt op
        nc.vector.scalar_tensor_tensor(
            ev.g[:, sl], ev.en[:, sl], 1.0, ev.rn[:, sl],
            op0=OP.subtract, op1=OP.mult,
        )

    def dve_out_pair(p, half=None):
        ev = blocks[2 * p]
        if getattr(ev, "outt", None) is None:
            ev.outt = psqp.tile([128, 2 * N * P8], f16, tag="outt", name="outtp")
        outt = ev.outt
        # (-s) * g = s * (1-en)/norm
        halves = (0, 1) if half is None else (half,)
        for h in halves:
            sl = slice(h * N * P8, (h + 1) * N * P8)
            if half is None and h == 0:
                sl = slice(0, 2 * N * P8)
            if half is None:
                if h == 1:
                    break
                nc.vector.scalar_tensor_tensor(
                    outt[:], ev.s_sbp[:], -1.0, ev.g[:], op0=OP.mult, op1=OP.mult
                )
            else:
                nc.vector.scalar_tensor_tensor(
                    outt[:, sl], ev.s_sbp[:, sl], -1.0, ev.g[:, sl],
                    op0=OP.mult, op1=OP.mult,
                )
            nc.scalar.dma_start(o_d[2 * p + h] if half is not None else o_d[2 * p], outt[:, 0 : N * P8] if half is None else outt[:, sl])
        if half is None:
            nc.scalar.dma_start(o_d[2 * p + 1], outt[:, N * P8 : 2 * N * P8])

    def dve_w_chunk(i, ck):
        """w(i) chunk ck=(half, ibpair): one wide op over 2 ib blocks,
        half-0 chunks first."""
        half, ibp = divmod(ck, 2)
        bl = blocks[i]
        if bl.w is None:
            bl.w = pw.tile([128, 4 * N * BLK], f16, tag="w", name="w_t")
        w4 = bl.w[:].rearrange("P (ib c) -> P ib c", ib=4)
        u4 = bl.u[:].rearrange("P (ib c) -> P ib c", ib=4)
        csl = slice(half * 1024, (half + 1) * 1024)
        ibsl = slice(ibp * 2, (ibp + 1) * 2)
        v_b = (
            bl.v_sb[:, csl].unsqueeze(1).broadcast_to([128, 2, 1024])
        )
        nc.vector.tensor_tensor(
            w4[:, ibsl, csl], u4[:, ibsl, csl], v_b, op=OP.mult
        )

    # ---- pipelined emission ----
    # Iteration i: PE [cp(i), v(i+1), cbc(i)⊗s(i-1), n2(i-1)];
    # DVE [squash-tail(i-2), rz(i), w(i+1)⊗m-units(i)]; Act/Pool follow.
    nc.sync.dma_start(wv_t[:], aps["wv"])
    dma_u(0, chunked=True)
    nc.sync.dma_start(wc_t[:], aps["wc"])
    nc.sync.dma_start(wcb_t[:], aps["wcb"])
    nc.sync.dma_start(ws_t[:], aps["ws"])
    nc.sync.dma_start(wn_t[:], aps["wn"])
    nc.sync.dma_start(gidx_t[:], aps["gidx"])
    if with_b:
        nc.sync.dma_start(bt_t[:], aps["bt"])
    dma_u(1, chunked="halves")
    if nblk > 2:
        dma_u(2)

    dve_vp(0, nchunks=4)
    pe_v(0)  # prologue
    for ck in range(4):
        dve_w_chunk(0, ck)

    for i in range(nblk + 3):
        has_cur = 0 <= i < nblk

        # presum for the next block first: dependency-free DVE work that
        # fills the engine while PE runs cp(i) and the gather(i-2) lands
        if i >= 1 and i + 1 < nblk:
            dve_vp(i + 1)

        # m-mults for block i-2's DMA-broadcast units (gather issued at the
        # end of iter i-2, landed during iter i-1); consumed by the sp2
        # matmuls of s(i-2) in this iteration's interleave.
        if 2 <= i <= nblk + 1:
            m_unit_dma_group(i - 2, 1, nc.vector)
            m_unit_dma_group(i - 2, 2, nc.vector)
            m_unit_dma_group(i - 2, 3, nc.gpsimd)

        # squash tail for pair p=(i-4)//2 (ln'd at end of iter i-1).
        # The last pair runs per-block halves so block nblk-2's tail
        # overlaps the final s-phase and block nblk-1's drain chain is short.
        if i >= 3 and i - 3 == nblk - 2 and nblk % 2 == 0:
            # even block of the final pair: its norm was ln'd last iter;
            # run its whole tail now, overlapped with the final s-phase
            p = (nblk - 2) // 2
            act_en_pair(p, half=0)
            dve_rn_pair(p, half=0)
            pool_g_pair(p, last=True, half=0)
            dve_out_pair(p, half=0)
        if i >= 4 and (i - 4) % 2 == 0 and i - 4 < nblk:
            p = (i - 4) // 2
            if 2 * p + 2 >= nblk:
                act_en_pair(p, half=1)
                dve_rn_pair(p, half=1)
                pool_g_pair(p, last=True, half=1)
                dve_out_pair(p, half=1)
            else:
                act_en_pair(p)
                dve_rn_pair(p)
                pool_g_pair(p, last=True)
                dve_out_pair(p)

        if has_cur:
            pe_cp(i)
            act_exp(i)
            soft(i)
            dma_cbc_bounce(i)
            if i + 3 < nblk:
                dma_u(i + 3)

        if i + 1 < nblk:
            if i == 0:
                dve_vp(1, nchunks=2)
            pe_v(i + 1)

        # main interleaved phase: cbc(i) units + s-matmuls (sp1 of block i-1,
        # sp2 of block i-2) + m(i) + w(i+1)
        w_chunks = list(range(4)) if i + 1 < nblk else []
        for u_ix in range(16):
            if u_ix < N_PE_CBC:
                if 0 <= i - 1 < nblk:
                    pe_s_unit(i - 1, u_ix)
            else:
                if 0 <= i - 2 < nblk:
                    pe_s_unit(i - 2, u_ix)
            if has_cur:
                if u_ix % 4 == 0 and w_chunks:
                    dve_w_chunk(i + 1, w_chunks.pop(0))
                if u_ix < N_PE_CBC:
                    cb = pe_cbc_unit(i, u_ix)
                    m_unit(i, u_ix, cb)
        for ck in w_chunks:
            dve_w_chunk(i + 1, ck)
        if has_cur:
            dma_cbc_gather(i)

        if 0 <= i - 1 < nblk:
            act_park1(i - 1)
        if 0 <= i - 2 < nblk:
            dve_stot(i - 2)
            act_square(i - 2)
            pe_n2(i - 2)
            if (i - 2) % 2 == 1:
                if i >= nblk:
                    # last pair: per-block ln (block i-3's half already done)
                    act_sqrt_pair((i - 2) // 2, half=1)
                else:
                    act_sqrt_pair((i - 2) // 2)
            elif i == nblk:
                # even block of the final pair: ln its half right away
                act_sqrt_pair((i - 2) // 2, half=0)


def encode_u(shard):
    """[I, N, J, pix] -> [nblk][(j,il) part, (ib, n, p64)] f16 device layout."""
    pix = shard.shape[-1]
    nblk = pix // BLK
    a = shard.reshape(4, 8, N, J, nblk, BLK)          # ib, il, n, j, blk, p
    # -> blk, j, il, ib, n, p
    return np.ascontiguousarray(a.transpose(4, 3, 1, 0, 2, 5)).astype(np.float16)


def decode_out(arr, pix):
    """[nblk, 128=(q8,j), N*P8] f16 device layout -> [N, J, pix] f32."""
    nblk = pix // BLK
    a = arr.astype(np.float32).reshape(nblk, 8, J, N, P8)
    return np.ascontiguousarray(a.transpose(3, 2, 0, 1, 4)).reshape(N, J, pix)


_CACHE = {}


def _patch_act_tables():
    """Keep only the act-table sets this kernel uses so a single table load is
    emitted instead of per-block set flip-flops."""
    if getattr(bacc, "_ant_act_tables_patched", False):
        return
    real = bacc.get_activation_tables

    def patched(module_arch):
        tabs = real(module_arch)
        keep = {"natural_log_exp_and_others"}
        return {
            name: (fns if name in keep else set())
            for name, fns in tabs.items()
        }

    bacc.get_activation_tables = patched
    bacc._ant_act_tables_patched = True


def _get_program(pix, with_b=False):
    key = (pix, with_b)
    if key in _CACHE:
        return _CACHE[key]
    _patch_act_tables()
    nc = bacc.Bacc("TRN2", target_bir_lowering=False, debug=False)
    # register the sqrt-bias constant (per-partition scalar AP)
    _eps_t = nc.alloc_sbuf_tensor("const-f32-eps30", [128, 1], f32)
    nc.gpsimd.memset(_eps_t.ap(), 1e-30)
    nc.const_aps.aps[(f32, 1e-30)] = _eps_t.ap()
    aps = {}
    nblk = pix // BLK
    aps["u"] = nc.dram_tensor(
        "u", [nblk, 128, 4 * N * BLK], f16, kind="ExternalInput"
    ).ap()
    wts = _build_weight_arrays()
    aps["wv"] = nc.dram_tensor("wv", [128, 128], f16, kind="ExternalInput").ap()
    aps["wc"] = nc.dram_tensor("wc", [128, 16 * 128], f16, kind="ExternalInput").ap()
    aps["wcb"] = nc.dram_tensor("wcb", [128, 4 * 128], f16, kind="ExternalInput").ap()
    aps["ws"] = nc.dram_tensor("ws", [128, 8 * 128], f16, kind="ExternalInput").ap()
    aps["wn"] = nc.dram_tensor("wn", [128, 128], f32r, kind="ExternalInput").ap()
    aps["gidx"] = nc.dram_tensor(
        "gidx", [128, (16 - N_PE_CBC) // 4 * 128 // 16], dt.int16, kind="ExternalInput"
    ).ap()
    aps["bt"] = nc.dram_tensor("bt", [128, N * P16], f32, kind="ExternalInput").ap()
    aps["out"] = nc.dram_tensor(
        "out", [nblk, 128, N * P8], f16, kind="ExternalOutput"
    ).ap()

    with tile.TileContext(nc) as tc:
        with ExitStack() as ctx:
            _emit(ctx, tc, aps, pix, with_b)
    nc.compile()

    _CACHE[key] = (nc, wts)
    return _CACHE[key]


def kernel(u: np.ndarray, b: np.ndarray) -> np.ndarray:
    u = np.asarray(u, dtype=np.float32)
    b = np.asarray(b, dtype=np.float32)
    with_b = bool(np.any(b))
    nc, wts = _get_program(PIX, with_b=with_b)

    base = {
        "wv": wts["wv"].astype(np.float16),
        "wc": wts["wc"].astype(np.float16),
        "wcb": wts["wcb"].astype(np.float16),
        "ws": wts["ws"].astype(np.float16),
        "wn": wts["wn"],
        "gidx": _gidx_array(),
        "bt": _b_tile_array(b),
    }
    in_maps = []
    for c in range(NCORES):
        bb = c // 2
        h0 = 16 * (c % 2)
        shard = u[bb, :, :, :, h0 : h0 + 16, :].reshape(I, N, J, PIX)
        m = dict(base)
        m["u"] = encode_u(shard)
        in_maps.append(m)

    res = run_bass_kernel_spmd(nc, in_maps, core_ids=list(range(NCORES)))
    out = np.zeros((B, N, J, H, W), np.float32)
    for c in range(NCORES):
        bb = c // 2
        h0 = 16 * (c % 2)
        out[bb, :, :, h0 : h0 + 16, :] = decode_out(
            res.results[c]["out"], PIX
        ).reshape(N, J, 16, W)
    return out

